# revision 23
# baseline (speedup 1.0000x reference)
"""Trainium2 Bass kernel: BFP-quantize -> 3x3 conv -> BatchNorm (batch stats) -> ReLU.

Full-input contract: kernel(x, W, gamma, beta) takes the complete arrays
(x [32,256,56,56] f32, W [256,256,3,3] OIHW f32, gamma/beta [256] f32) and
returns the full [32,256,56,56] f32 output.

Distribution: data-parallel over batch, 4 images per core across 8 cores.
BatchNorm statistics (per-channel sum / sum-of-squares) are all-reduced
across the cores; every core then applies the identical normalization to its
batch shard.

Per-core pipeline (v2 — restructured from the 379us baseline):
  1. Image DMA into a zero-padded [58,58] SBUF buffer; xpad is double-
     buffered (2 phases) so image N+1's DMA overlaps image N's quantize.
     Image 0's DMA is split into 4 row bands so quantization starts early.
  2. BFP block-quantize in quarter windows, work spread across engines:
     DVE 32x32 stream-transpose + abs-max reduce + magic-number round +
     back-transpose; GPSIMD scale-mult and clip; per-cin consolidated
     exponent math via IEEE bit tricks.  Quantized activations stored bf16.
  3. Conv = 9 shifted bf16 matmuls x 2 cin-halves accumulated in PSUM,
     8-row chunks (N=448, one PSUM bank), chunk-pair groups.
  4. PSUM->SBUF fp16 copy of y on ScalarE; BN stats via one bn_stats per
     (img, ch) over fp16 y for imgs 0-2, per-chunk fp32 PSUM stats for the
     last image (keeps the tail short).
  5. bn_aggr -> per-core (sum, sumsq) [P,4] -> AllReduce (a tiny warmup
     AllReduce early in the kernel primes the CC path) -> global mean/var
     with both channel halves vectorized -> scale/shift.
  6. Apply relu(y*s+t): ScalarE fused activation for 5 image-halves, DVE
     max-trick (relu(s*y+t) = s*max(y,-t/s)+t) for 3; bf16 output DMA.
"""

import sys

for _p in ("/opt/trn_rl_repo",):
    if _p not in sys.path:
        sys.path.insert(0, _p)

import numpy as np
import ml_dtypes

from concourse import bass, bacc, tile, mybir
from concourse.bass_utils import run_bass_kernel_spmd

F32 = mybir.dt.float32
BF16 = mybir.dt.bfloat16
FP16 = mybir.dt.float16
I32 = mybir.dt.int32

P = 128
H = W_SP = 56
HP = 58                      # padded row length
SPATIAL = H * W_SP           # 3136
PADLEN = 3368                # 58*58 = 3364 rounded up so tap APs stay in-bounds
QW0, QW1 = 32, 3328          # 32-aligned quantize window covering all data rows
QLEN = QW1 - QW0             # 3296 = 32*103
NBLK = QLEN // 32            # 103
CIN_T = 2                    # 256 channels = 2 partition tiles
COUT_H = 2
TAPS = 9
ROWS_PER_CHUNK = 8
NCHUNK = H // ROWS_PER_CHUNK          # 7
CHUNK_N = ROWS_PER_CHUNK * W_SP       # 448
MAGIC = float(1.5 * 2.0**23)
EXP_MASK = 0x7F800000
EXP_RSUB = float(0x7F000000)          # 2^-e bits = 0x7F000000 - 2^e bits

QUARTERS = [832, 832, 832, 800]       # 26+26+26+25 blocks of 32
QOFFS = [0, 832, 1664, 2496]
QBLK = [26, 26, 26, 25]
QBOFF = [0, 26, 52, 78]
# img-0 DMA row bands sized so quantize quarter q only needs bands <= q
BANDS = [(0, 14), (14, 29), (29, 43), (43, 56)]


def build_program(n_cores: int, imgs_per_core: int):
    nc = bacc.Bacc(
        "TRN2", target_bir_lowering=False, debug=False, num_devices=n_cores
    )
    B = imgs_per_core
    x_d = nc.dram_tensor("x", [B, 256, H, W_SP], F32, kind="ExternalInput")
    wt_d = nc.dram_tensor("wt", [TAPS, CIN_T, P, 256], BF16, kind="ExternalInput")
    gb_d = nc.dram_tensor("gb", [P, 4], F32, kind="ExternalInput")
    out_d = nc.dram_tensor("out", [B, 256, H, W_SP], BF16, kind="ExternalOutput")

    n_count = float(B * SPATIAL)              # per-core samples per channel
    n_total = float(n_cores * B * SPATIAL)    # global samples per channel

    with tile.TileContext(nc) as tc:
        with (
            tc.tile_pool(name="persist", bufs=1) as pp,
            tc.tile_pool(name="xpad", bufs=1) as xpadp,
            tc.tile_pool(name="xqpad", bufs=1) as xqp,
            tc.tile_pool(name="qf32", bufs=8) as qf,
            tc.tile_pool(name="qbf", bufs=3) as qb,
            tc.tile_pool(name="small", bufs=8) as sm,
            tc.tile_pool(name="tiny", bufs=24) as tp,
            tc.tile_pool(name="ostage", bufs=3) as op_,
            tc.tile_pool(name="psum", bufs=8, space="PSUM") as ps_pool,
            tc.tile_pool(name="dram", bufs=4, space="DRAM") as dramp,
        ):
            NPHASE = 2
            # ---- image-0 DMA first (head latency) ----
            xpad = [
                [xpadp.tile([P, PADLEN], F32, tag=f"xp{phz}_{ct}", name=f"xpad{phz}_{ct}")
                 for ct in range(CIN_T)]
                for phz in range(NPHASE)
            ]

            def dst_rows(t, r0, r1):
                # padded-image rows r0+1..r1 (image rows r0..r1-1), cols 1..56
                return t[:, (r0 + 1) * HP : (r0 + 1) * HP + (r1 - r0) * HP].rearrange(
                    "p (r w) -> p r w", r=r1 - r0
                )[:, :, 1 : 1 + W_SP]

            for r0, r1 in BANDS:
                for ct in range(CIN_T):
                    nc.sync.dma_start(
                        out=dst_rows(xpad[0][ct], r0, r1),
                        in_=x_d.ap()[0, ct * P : (ct + 1) * P, r0:r1].rearrange(
                            "c h w -> c (h w)"
                        ),
                    )

            # ---- persistent tiles ----
            wsb = pp.tile([P, TAPS * CIN_T * 256], BF16, tag="wsb")
            nc.sync.dma_start(
                out=wsb[:].rearrange("p (t k o) -> p t k o", t=TAPS, k=CIN_T),
                in_=wt_d.ap().transpose([2, 0, 1, 3]),
            )
            wv = wsb[:].rearrange("p (t k o) -> p t k o", t=TAPS, k=CIN_T)

            gbsb = pp.tile([P, 4], F32, tag="gbsb")
            nc.sync.dma_start(out=gbsb[:], in_=gb_d.ap())

            ybuf = [
                pp.tile([P, B * SPATIAL], FP16, tag=f"y{ch}", name=f"ybuf{ch}")
                for ch in range(COUT_H)
            ]
            # per-chunk partial sums / sums-of-squares (ScalarE accum_out)
            NCH = B * NCHUNK
            ssum = [
                pp.tile([P, NCH], F32, tag=f"ssum{ch}", name=f"ssum{ch}")
                for ch in range(COUT_H)
            ]
            ssq = [
                pp.tile([P, NCH], F32, tag=f"ssq{ch}", name=f"ssq{ch}")
                for ch in range(COUT_H)
            ]

            xq = [
                [xqp.tile([P, PADLEN], BF16, tag=f"xq{phz}_{ct}", name=f"xqpad{phz}_{ct}")
                 for ct in range(CIN_T)]
                for phz in range(NPHASE)
            ]
            for phz in range(NPHASE):
                for t in xpad[phz]:
                    # zero only the pad positions (head row + per-row col pairs
                    # + tail); the interior is overwritten by every image DMA
                    nc.gpsimd.memset(t[:, 0:59], 0.0)
                    nc.gpsimd.memset(
                        t[:, 115:115 + 55 * HP].rearrange(
                            "p (r w) -> p r w", r=55
                        )[:, :, 0:2],
                        0.0,
                    )
                    nc.gpsimd.memset(t[:, 3305:PADLEN], 0.0)
                for t in xq[phz]:
                    nc.gpsimd.memset(t[:, :QW0], 0.0)
                    nc.gpsimd.memset(t[:, QW1:], 0.0)

            # warmup: tiny AllReduce to prime the CC stream + sqrt ACT table
            warm = tp.tile([P, 1], F32, tag="t1", name="warm")
            nc.scalar.activation(
                warm[:], gbsb[:, 0:1], mybir.ActivationFunctionType.Sqrt
            )
            cc_w_in = dramp.tile([P, 1], F32)
            cc_w_out = dramp.tile([P, 1], F32)
            nc.sync.dma_start(out=cc_w_in[:], in_=gbsb[:, 0:1])
            nc.gpsimd.collective_compute(
                "AllReduce",
                mybir.AluOpType.add,
                replica_groups=[list(range(n_cores))],
                ins=[cc_w_in[:].opt()],
                outs=[cc_w_out[:].opt()],
            )

            QPAIRS = [(0, 1), (2, 3)]

            def emit_quantize(img):
                """BFP-quantize image `img` from xpad[phase] into xq[phase].

                For pipelined images (img > 0) the per-quarter chains are
                emitted stage-major: each engine's FIFO then matches the
                dependency order across quarters (assembly line), avoiding
                head-of-line blocking when the queues are deep.  img 0 runs
                on empty queues where chain-major has lower latency.
                """
                stage_major = img > 0
                phz = img % NPHASE
                S, inv2, pes = [], [], []
                for ct in range(CIN_T):
                    S.append(sm.tile([P, NBLK], F32, tag="sv", name=f"qS{ct}"))
                    inv2.append(sm.tile([P, NBLK], F32, tag="sv", name=f"qi{ct}"))
                    pes.append(sm.tile([P, NBLK], F32, tag="sv", name=f"qp{ct}"))
                Ts = {}
                for pair in QPAIRS:
                    b0 = QBOFF[pair[0]]
                    bn = QBOFF[pair[-1]] + QBLK[pair[-1]] - b0
                    ssl = slice(b0, b0 + bn)
                    for ct in range(CIN_T):
                        xp = xpad[phz][ct]
                        for q in pair:
                            w0 = QW0 + QOFFS[q]
                            wlen = QUARTERS[q]
                            T = qf.tile([P, wlen], F32, tag="q", name="qT")
                            nc.vector.transpose(T[:], xp[:, w0 : w0 + wlen])
                            nc.vector.tensor_reduce(
                                S[ct][:, QBOFF[q] : QBOFF[q] + QBLK[q]],
                                T[:].rearrange("p (b k) -> p b k", k=32),
                                axis=mybir.AxisListType.X,
                                op=mybir.AluOpType.max,
                                apply_absolute_value=True,
                            )
                            Ts[(ct, q)] = T
                        # exponent math for this quarter-pair on [P, bn]
                        m = sm.tile([P, bn], F32, tag="st", name="qm")
                        nc.vector.tensor_scalar(
                            m[:], S[ct][:, ssl], 1e-12, None,
                            op0=mybir.AluOpType.max,
                        )
                        peb = sm.tile([P, bn], I32, tag="st", name="qpeb")
                        nc.vector.tensor_scalar(
                            peb[:], m[:].bitcast(I32), EXP_MASK, None,
                            op0=mybir.AluOpType.bitwise_and,
                        )
                        invb = sm.tile([P, bn], I32, tag="st", name="qinvb")
                        nc.vector.tensor_scalar(
                            invb[:], peb[:], EXP_RSUB, -1.0,
                            op0=mybir.AluOpType.subtract, op1=mybir.AluOpType.mult,
                        )
                        nc.vector.tensor_scalar(
                            inv2[ct][:, ssl], invb[:].bitcast(F32), 128.0, None,
                            op0=mybir.AluOpType.mult,
                        )
                        nc.vector.tensor_scalar(
                            pes[ct][:, ssl], peb[:].bitcast(F32), 0.0078125, None,
                            op0=mybir.AluOpType.mult,
                        )
                    # separate tiles per stage: in-place (same AP in/out)
                    # measures 2-4x slower — it defeats the DVE/GPSIMD
                    # perf modes
                    if stage_major:
                        chain = [(ct, q) for ct in range(CIN_T) for q in pair]
                    else:
                        # img 0: q-major so both cin-halves of the earlier
                        # quarter finish first (conv groups consume quarters
                        # in order)
                        chain = [(ct, q) for q in pair for ct in range(CIN_T)]
                    Vs, R2s, Cs, Qs = {}, {}, {}, {}

                    def st_v(ct, q):
                        wlen, nb = QUARTERS[q], QBLK[q]
                        bsl = slice(QBOFF[q], QBOFF[q] + nb)
                        v = qf.tile([P, wlen], F32, tag="q", name="qv")
                        nc.gpsimd.tensor_tensor(
                            out=v[:].rearrange("p (b k) -> p b k", k=32),
                            in0=Ts[(ct, q)][:].rearrange("p (b k) -> p b k", k=32),
                            in1=inv2[ct][:, bsl].unsqueeze(2).to_broadcast(
                                (P, nb, 32)
                            ),
                            op=mybir.AluOpType.mult,
                        )
                        Vs[(ct, q)] = v

                    def st_r2(ct, q):
                        r2 = qf.tile([P, QUARTERS[q]], F32, tag="q", name="qr2")
                        nc.vector.tensor_scalar(
                            r2[:], Vs[(ct, q)][:], MAGIC, -MAGIC,
                            op0=mybir.AluOpType.add, op1=mybir.AluOpType.add,
                        )
                        R2s[(ct, q)] = r2

                    def st_c(ct, q):
                        c = qf.tile([P, QUARTERS[q]], F32, tag="q", name="qc")
                        nc.gpsimd.tensor_scalar(
                            c[:], R2s[(ct, q)][:], 127.0, -128.0,
                            op0=mybir.AluOpType.min, op1=mybir.AluOpType.max,
                        )
                        Cs[(ct, q)] = c

                    def st_qT(ct, q):
                        wlen, nb = QUARTERS[q], QBLK[q]
                        bsl = slice(QBOFF[q], QBOFF[q] + nb)
                        qT = qb.tile([P, wlen], BF16, tag="qb", name="qq")
                        eng = nc.vector if q % 2 == 0 else nc.gpsimd
                        eng.tensor_tensor(
                            out=qT[:].rearrange("p (b k) -> p b k", k=32),
                            in0=Cs[(ct, q)][:].rearrange("p (b k) -> p b k", k=32),
                            in1=pes[ct][:, bsl].unsqueeze(2).to_broadcast(
                                (P, nb, 32)
                            ),
                            op=mybir.AluOpType.mult,
                        )
                        Qs[(ct, q)] = qT

                    def st_out(ct, q):
                        w0 = QW0 + QOFFS[q]
                        nc.vector.transpose(
                            xq[phz][ct][:, w0 : w0 + QUARTERS[q]], Qs[(ct, q)][:]
                        )

                    stages = (st_v, st_r2, st_c, st_qT, st_out)
                    if stage_major:
                        for f in stages:
                            for ct, q in chain:
                                f(ct, q)
                    else:
                        for ct, q in chain:
                            for f in stages:
                                f(ct, q)

            def emit_dma(img):
                phz = img % NPHASE
                for ct in range(CIN_T):
                    nc.sync.dma_start(
                        out=dst_rows(xpad[phz][ct], 0, H),
                        in_=x_d.ap()[img, ct * P : (ct + 1) * P].rearrange(
                            "c h w -> c (h w)"
                        ),
                    )

            # ---- conv ----
            # img 0: small groups, group-outer/channel-inner, so early
            # groups only need the first quantize quarter-pair and late
            # groups buy time for pair 1.  imgs 1+: 4-chunk groups amortize
            # weight switches across 4 matmuls.
            GROUPS_HEAD = [(0, 1), (2, 3), (4, 5), (6,)]
            GROUPS_STEADY = [(0, 1, 2, 3), (4, 5, 6)]

            USE_CRIT = True  # critical-section matmul blocks (no per-MM sems)

            def emit_conv(img):
                phz = img % NPHASE
                groups = GROUPS_HEAD if img == 0 else GROUPS_STEADY
                for grp in groups:
                    for ch in range(COUT_H):
                        pss = {
                            chunk: ps_pool.tile(
                                [P, CHUNK_N], F32, tag="ps", name=f"ps{chunk}"
                            )
                            for chunk in grp
                        }

                        def mm_block():
                            for kt in range(CIN_T):
                                for tap in range(TAPS):
                                    kh, kw = divmod(tap, 3)
                                    acc_i = kt * TAPS + tap
                                    lhsT = wv[:, tap, kt, ch * P : (ch + 1) * P]
                                    for chunk in grp:
                                        base = (
                                            chunk * ROWS_PER_CHUNK + kh
                                        ) * HP + kw
                                        rhs = (
                                            xq[phz][kt][
                                                :, base : base + ROWS_PER_CHUNK * HP
                                            ]
                                            .rearrange(
                                                "p (r w) -> p r w",
                                                r=ROWS_PER_CHUNK,
                                            )[:, :, :W_SP]
                                        )
                                        nc.tensor.matmul(
                                            pss[chunk][:],
                                            lhsT,
                                            rhs,
                                            start=(acc_i == 0),
                                            stop=(acc_i == 2 * TAPS - 1),
                                        )

                        if USE_CRIT and img > 0:
                            with tc.tile_critical(name=f"mm{img}_{ch}"):
                                mm_block()
                        else:
                            mm_block()
                        for chunk in grp:
                            idx = img * NCHUNK + chunk
                            ysl = ybuf[ch][
                                :, img * SPATIAL + chunk * CHUNK_N :
                                img * SPATIAL + (chunk + 1) * CHUNK_N
                            ]
                            nc.scalar.activation(
                                ysl, pss[chunk][:],
                                mybir.ActivationFunctionType.Copy,
                                accum_out=ssum[ch][:, idx : idx + 1],
                            )
                            scr = sm.tile([P, CHUNK_N], BF16, tag="scr", name="sq")
                            nc.scalar.activation(
                                scr[:], pss[chunk][:],
                                mybir.ActivationFunctionType.Square,
                                accum_out=ssq[ch][:, idx : idx + 1],
                            )

            # ---- main loop (software-pipelined) ----
            emit_quantize(0)
            for img in range(B):
                if img + 1 < B:
                    emit_dma(img + 1)
                    emit_quantize(img + 1)
                emit_conv(img)

            # ---- BN statistics reduce ----
            # layout: [sum_ch0, sum_ch1, sumsq_ch0, sumsq_ch1]
            sums_all = pp.tile([P, 4], F32, tag="sums_all")
            for ch in range(COUT_H):
                nc.vector.tensor_reduce(
                    sums_all[:, ch : ch + 1], ssum[ch][:],
                    axis=mybir.AxisListType.X, op=mybir.AluOpType.add,
                )
                nc.vector.tensor_reduce(
                    sums_all[:, 2 + ch : 3 + ch], ssq[ch][:],
                    axis=mybir.AxisListType.X, op=mybir.AluOpType.add,
                )
            gsum = tp.tile([P, 4], F32, tag="t4", name="gsum")
            cc_in = dramp.tile([P, 4], F32)
            cc_out = dramp.tile([P, 4], F32)
            nc.sync.dma_start(out=cc_in[:], in_=sums_all[:])
            nc.gpsimd.collective_compute(
                "AllReduce",
                mybir.AluOpType.add,
                replica_groups=[list(range(n_cores))],
                ins=[cc_in[:].opt()],
                outs=[cc_out[:].opt()],
            )
            nc.sync.dma_start(out=gsum[:], in_=cc_out[:])

            # ---- global scale/shift, both channel halves at once [P,2] ----
            gmean = tp.tile([P, 2], F32, tag="t2")
            nc.vector.tensor_scalar(
                gmean[:], gsum[:, 0:2], 1.0 / n_total, None,
                op0=mybir.AluOpType.mult,
            )
            gex2 = tp.tile([P, 2], F32, tag="t2")
            nc.vector.tensor_scalar(
                gex2[:], gsum[:, 2:4], 1.0 / n_total, None,
                op0=mybir.AluOpType.mult,
            )
            gm2 = tp.tile([P, 2], F32, tag="t2")
            nc.vector.tensor_tensor(gm2[:], gmean[:], gmean[:], op=mybir.AluOpType.mult)
            vdiff = tp.tile([P, 2], F32, tag="t2")
            nc.vector.tensor_tensor(
                vdiff[:], gex2[:], gm2[:], op=mybir.AluOpType.subtract
            )
            veps = tp.tile([P, 2], F32, tag="t2")  # var + eps
            nc.vector.tensor_scalar(
                veps[:], vdiff[:], 1e-5, None, op0=mybir.AluOpType.add
            )
            rec = tp.tile([P, 2], F32, tag="t2")  # 1/(var+eps)
            nc.vector.reciprocal(rec[:], veps[:])
            s0 = tp.tile([P, 2], F32, tag="t2")   # ~= 1/sqrt(var+eps)
            nc.scalar.activation(s0[:], rec[:], mybir.ActivationFunctionType.Sqrt)
            # one Newton step: s1 = s0 * (1.5 - 0.5 * veps * s0^2)
            a = tp.tile([P, 2], F32, tag="t2")
            nc.vector.tensor_tensor(a[:], s0[:], s0[:], op=mybir.AluOpType.mult)
            b = tp.tile([P, 2], F32, tag="t2")
            nc.vector.tensor_tensor(b[:], a[:], veps[:], op=mybir.AluOpType.mult)
            bb = tp.tile([P, 2], F32, tag="t2")
            nc.vector.tensor_scalar(
                bb[:], b[:], -0.5, 1.5,
                op0=mybir.AluOpType.mult, op1=mybir.AluOpType.add,
            )
            s1 = tp.tile([P, 2], F32, tag="t2")
            nc.vector.tensor_tensor(s1[:], s0[:], bb[:], op=mybir.AluOpType.mult)
            scale = tp.tile([P, 2], F32, tag="sc", name="scale")
            nc.vector.tensor_tensor(
                scale[:], s1[:], gbsb[:, 0:2], op=mybir.AluOpType.mult
            )
            t2 = tp.tile([P, 2], F32, tag="t2")
            nc.vector.tensor_tensor(t2[:], gmean[:], scale[:], op=mybir.AluOpType.mult)
            shift = tp.tile([P, 2], F32, tag="sc", name="shift")
            nc.vector.tensor_tensor(
                shift[:], gbsb[:, 2:4], t2[:], op=mybir.AluOpType.subtract
            )
            # u = -shift/scale for the DVE relu max-trick
            rs = tp.tile([P, 2], F32, tag="t2")
            nc.vector.reciprocal(rs[:], scale[:])
            nshift = tp.tile([P, 2], F32, tag="t2")
            nc.vector.tensor_scalar(
                nshift[:], shift[:], -1.0, None, op0=mybir.AluOpType.mult
            )
            u = tp.tile([P, 2], F32, tag="sc", name="uthr")
            nc.vector.tensor_tensor(u[:], nshift[:], rs[:], op=mybir.AluOpType.mult)

            # ---- apply BN + ReLU, write out (bf16) ----
            # DVE takes 5 image-halves via the max-trick (fp16 2x mode,
            # ~2.2us each); ScalarE takes 3 (~3.0us each).
            DVE_APPLY = {(0, 0), (0, 1), (1, 0), (1, 1), (2, 0)}
            for img in range(B):
                for ch in range(COUT_H):
                    ysl = ybuf[ch][:, img * SPATIAL : (img + 1) * SPATIAL]
                    o = op_.tile([P, SPATIAL], BF16, tag="o", name="ostage")
                    if (img, ch) in DVE_APPLY:
                        # s*max(y, u) in place into the dead ybuf slice
                        # (s > 0 since gamma > 0), then + shift
                        nc.vector.tensor_scalar(
                            ysl, ysl, u[:, ch : ch + 1], scale[:, ch : ch + 1],
                            op0=mybir.AluOpType.max, op1=mybir.AluOpType.mult,
                        )
                        nc.vector.tensor_scalar(
                            o[:], ysl, shift[:, ch : ch + 1], None,
                            op0=mybir.AluOpType.add,
                        )
                    else:
                        nc.scalar.activation(
                            o[:], ysl,
                            mybir.ActivationFunctionType.Relu,
                            bias=shift[:, ch : ch + 1],
                            scale=scale[:, ch : ch + 1],
                        )
                    nc.sync.dma_start(
                        out=out_d.ap()[img, ch * P : (ch + 1) * P].rearrange(
                            "c h w -> c (h w)"
                        ),
                        in_=o[:],
                    )

    nc.compile()
    return nc


def host_prep(W, gamma, beta):
    # lhsT layout per tap: [cin, cout];  wt[t, kt, p, o] = W[o, kt*128+p, kh, kw]
    wt = np.ascontiguousarray(
        W.transpose(2, 3, 1, 0).reshape(TAPS, CIN_T, P, 256)
    ).astype(ml_dtypes.bfloat16)
    gb = np.empty((P, 4), np.float32)
    gb[:, 0] = gamma[:P]
    gb[:, 1] = gamma[P:]
    gb[:, 2] = beta[:P]
    gb[:, 3] = beta[P:]
    return wt, gb


_cache = {}


def _get_program(n_cores, imgs_per_core):
    key = (n_cores, imgs_per_core)
    if key not in _cache:
        _cache[key] = build_program(n_cores, imgs_per_core)
    return _cache[key]


def run(x, W, gamma, beta, n_cores=8, trace=False):
    B = x.shape[0]
    imgs_per_core = B // n_cores
    assert imgs_per_core * n_cores == B
    nc = _get_program(n_cores, imgs_per_core)
    wt, gb = host_prep(W, gamma, beta)
    in_maps = [
        {
            "x": np.ascontiguousarray(
                x[c * imgs_per_core : (c + 1) * imgs_per_core]
            ),
            "wt": wt,
            "gb": gb,
        }
        for c in range(n_cores)
    ]
    res = run_bass_kernel_spmd(nc, in_maps, list(range(n_cores)), trace=trace)
    out = np.concatenate(
        [res.results[c]["out"].astype(np.float32) for c in range(n_cores)], axis=0
    )
    return out, res


def kernel(x, W, gamma, beta):
    out, _ = run(
        np.asarray(x, np.float32),
        np.asarray(W, np.float32),
        np.asarray(gamma, np.float32),
        np.asarray(beta, np.float32),
    )
    return out


# revision 25
# speedup vs baseline: 1.3938x; 1.3938x over previous
"""Trainium2 Bass kernel: BFP-quantize -> 3x3 conv -> BatchNorm (batch stats) -> ReLU.

Full-input contract: kernel(x, W, gamma, beta) takes the complete arrays
(x [32,256,56,56] f32, W [256,256,3,3] OIHW f32, gamma/beta [256] f32) and
returns the full [32,256,56,56] f32 output.

Distribution: data-parallel over batch, 4 images per core across 8 cores.
BatchNorm statistics (per-channel sum / sum-of-squares) are all-reduced
across the cores; every core then applies the identical normalization to its
batch shard.

Per-core pipeline (v2 — restructured from the 379us baseline):
  1. Image DMA into a zero-padded [58,58] SBUF buffer; xpad is double-
     buffered (2 phases) so image N+1's DMA overlaps image N's quantize.
     Image 0's DMA is split into 4 row bands so quantization starts early.
  2. BFP block-quantize in quarter windows, work spread across engines:
     DVE 32x32 stream-transpose + abs-max reduce + magic-number round +
     back-transpose; GPSIMD scale-mult and clip; per-cin consolidated
     exponent math via IEEE bit tricks.  Quantized activations stored bf16.
  3. Conv = 9 shifted bf16 matmuls x 2 cin-halves accumulated in PSUM,
     8-row chunks (N=448, one PSUM bank), chunk-pair groups.
  4. PSUM->SBUF fp16 copy of y on ScalarE; BN stats via one bn_stats per
     (img, ch) over fp16 y for imgs 0-2, per-chunk fp32 PSUM stats for the
     last image (keeps the tail short).
  5. bn_aggr -> per-core (sum, sumsq) [P,4] -> AllReduce (a tiny warmup
     AllReduce early in the kernel primes the CC path) -> global mean/var
     with both channel halves vectorized -> scale/shift.
  6. Apply relu(y*s+t): ScalarE fused activation for 5 image-halves, DVE
     max-trick (relu(s*y+t) = s*max(y,-t/s)+t) for 3; bf16 output DMA.
"""

import sys

for _p in ("/opt/trn_rl_repo",):
    if _p not in sys.path:
        sys.path.insert(0, _p)

import numpy as np
import ml_dtypes

from concourse import bass, bacc, tile, mybir
from concourse.bass_utils import run_bass_kernel_spmd

F32 = mybir.dt.float32
BF16 = mybir.dt.bfloat16
FP16 = mybir.dt.float16
I32 = mybir.dt.int32

P = 128
H = W_SP = 56
HP = 58                      # padded row length
SPATIAL = H * W_SP           # 3136
PADLEN = 3368                # 58*58 = 3364 rounded up so tap APs stay in-bounds
QW0, QW1 = 32, 3328          # 32-aligned quantize window covering all data rows
QLEN = QW1 - QW0             # 3296 = 32*103
NBLK = QLEN // 32            # 103
CIN_T = 2                    # 256 channels = 2 partition tiles
COUT_H = 2
TAPS = 9
ROWS_PER_CHUNK = 8
NCHUNK = H // ROWS_PER_CHUNK          # 7
CHUNK_N = ROWS_PER_CHUNK * W_SP       # 448
MAGIC = float(1.5 * 2.0**23)
EXP_MASK = 0x7F800000
EXP_RSUB = float(0x7F000000)          # 2^-e bits = 0x7F000000 - 2^e bits

QUARTERS = [832, 832, 832, 800]       # 26+26+26+25 blocks of 32
QOFFS = [0, 832, 1664, 2496]
QBLK = [26, 26, 26, 25]
QBOFF = [0, 26, 52, 78]
# img-0 DMA row bands sized so quantize quarter q only needs bands <= q
BANDS = [(0, 14), (14, 29), (29, 43), (43, 56)]


def build_program(n_cores: int, imgs_per_core: int):
    nc = bacc.Bacc(
        "TRN2", target_bir_lowering=False, debug=False, num_devices=n_cores
    )
    B = imgs_per_core
    x_d = nc.dram_tensor("x", [B, 256, H, W_SP], F32, kind="ExternalInput")
    wt_d = nc.dram_tensor("wt", [TAPS, CIN_T, P, 256], BF16, kind="ExternalInput")
    gb_d = nc.dram_tensor("gb", [P, 4], F32, kind="ExternalInput")
    out_d = nc.dram_tensor("out", [B, 256, H, W_SP], BF16, kind="ExternalOutput")

    n_count = float(B * SPATIAL)              # per-core samples per channel
    n_total = float(n_cores * B * SPATIAL)    # global samples per channel

    with tile.TileContext(nc) as tc:
        with (
            tc.tile_pool(name="persist", bufs=1) as pp,
            tc.tile_pool(name="xpad", bufs=1) as xpadp,
            tc.tile_pool(name="xqpad", bufs=1) as xqp,
            tc.tile_pool(name="qf32", bufs=8) as qf,
            tc.tile_pool(name="qbf", bufs=3) as qb,
            tc.tile_pool(name="small", bufs=8) as sm,
            tc.tile_pool(name="tiny", bufs=24) as tp,
            tc.tile_pool(name="ostage", bufs=3) as op_,
            tc.tile_pool(name="psum", bufs=8, space="PSUM") as ps_pool,
            tc.tile_pool(name="dram", bufs=4, space="DRAM") as dramp,
        ):
            NPHASE = 2
            # ---- image-0 DMA first (head latency) ----
            xpad = [
                [xpadp.tile([P, PADLEN], F32, tag=f"xp{phz}_{ct}", name=f"xpad{phz}_{ct}")
                 for ct in range(CIN_T)]
                for phz in range(NPHASE)
            ]

            def dst_rows(t, r0, r1):
                # padded-image rows r0+1..r1 (image rows r0..r1-1), cols 1..56
                return t[:, (r0 + 1) * HP : (r0 + 1) * HP + (r1 - r0) * HP].rearrange(
                    "p (r w) -> p r w", r=r1 - r0
                )[:, :, 1 : 1 + W_SP]

            for r0, r1 in BANDS:
                for ct in range(CIN_T):
                    nc.sync.dma_start(
                        out=dst_rows(xpad[0][ct], r0, r1),
                        in_=x_d.ap()[0, ct * P : (ct + 1) * P, r0:r1].rearrange(
                            "c h w -> c (h w)"
                        ),
                    )

            # ---- persistent tiles ----
            wsb = pp.tile([P, TAPS * CIN_T * 256], BF16, tag="wsb")
            nc.sync.dma_start(
                out=wsb[:].rearrange("p (t k o) -> p t k o", t=TAPS, k=CIN_T),
                in_=wt_d.ap().transpose([2, 0, 1, 3]),
            )
            wv = wsb[:].rearrange("p (t k o) -> p t k o", t=TAPS, k=CIN_T)

            gbsb = pp.tile([P, 4], F32, tag="gbsb")
            nc.sync.dma_start(out=gbsb[:], in_=gb_d.ap())

            ybuf = [
                pp.tile([P, B * SPATIAL], FP16, tag=f"y{ch}", name=f"ybuf{ch}")
                for ch in range(COUT_H)
            ]
            # per-chunk partial sums / sums-of-squares (ScalarE accum_out)
            NCH = B * NCHUNK
            ssum = [
                pp.tile([P, NCH], F32, tag=f"ssum{ch}", name=f"ssum{ch}")
                for ch in range(COUT_H)
            ]
            ssq = [
                pp.tile([P, NCH], F32, tag=f"ssq{ch}", name=f"ssq{ch}")
                for ch in range(COUT_H)
            ]

            xq = [
                [xqp.tile([P, PADLEN], BF16, tag=f"xq{phz}_{ct}", name=f"xqpad{phz}_{ct}")
                 for ct in range(CIN_T)]
                for phz in range(NPHASE)
            ]
            for phz in range(NPHASE):
                for t in xpad[phz]:
                    # zero only the pad positions (head row + per-row col pairs
                    # + tail); the interior is overwritten by every image DMA
                    nc.gpsimd.memset(t[:, 0:59], 0.0)
                    nc.gpsimd.memset(
                        t[:, 115:115 + 55 * HP].rearrange(
                            "p (r w) -> p r w", r=55
                        )[:, :, 0:2],
                        0.0,
                    )
                    nc.gpsimd.memset(t[:, 3305:PADLEN], 0.0)
                for t in xq[phz]:
                    nc.gpsimd.memset(t[:, :QW0], 0.0)
                    nc.gpsimd.memset(t[:, QW1:], 0.0)

            # warmup: tiny AllReduce to prime the CC stream + sqrt ACT table
            warm = tp.tile([P, 1], F32, tag="t1", name="warm")
            nc.scalar.activation(
                warm[:], gbsb[:, 0:1], mybir.ActivationFunctionType.Sqrt
            )
            cc_w_in = dramp.tile([P, 1], F32)
            cc_w_out = dramp.tile([P, 1], F32)
            nc.sync.dma_start(out=cc_w_in[:], in_=gbsb[:, 0:1])
            nc.gpsimd.collective_compute(
                "AllReduce",
                mybir.AluOpType.add,
                replica_groups=[list(range(n_cores))],
                ins=[cc_w_in[:].opt()],
                outs=[cc_w_out[:].opt()],
            )

            QPAIRS = [(0, 1), (2, 3)]

            def emit_quantize(img):
                """BFP-quantize image `img` from xpad[phase] into xq[phase].

                For pipelined images (img > 0) the per-quarter chains are
                emitted stage-major: each engine's FIFO then matches the
                dependency order across quarters (assembly line), avoiding
                head-of-line blocking when the queues are deep.  img 0 runs
                on empty queues where chain-major has lower latency.
                """
                stage_major = img > 0
                phz = img % NPHASE
                S, inv2, pes = [], [], []
                for ct in range(CIN_T):
                    S.append(sm.tile([P, NBLK], F32, tag="sv", name=f"qS{ct}"))
                    inv2.append(sm.tile([P, NBLK], F32, tag="sv", name=f"qi{ct}"))
                    pes.append(sm.tile([P, NBLK], F32, tag="sv", name=f"qp{ct}"))
                Ts = {}
                for pair in QPAIRS:
                    b0 = QBOFF[pair[0]]
                    bn = QBOFF[pair[-1]] + QBLK[pair[-1]] - b0
                    ssl = slice(b0, b0 + bn)
                    for ct in range(CIN_T):
                        xp = xpad[phz][ct]
                        for q in pair:
                            w0 = QW0 + QOFFS[q]
                            wlen = QUARTERS[q]
                            T = qf.tile([P, wlen], F32, tag="q", name="qT")
                            nc.vector.transpose(T[:], xp[:, w0 : w0 + wlen])
                            nc.vector.tensor_reduce(
                                S[ct][:, QBOFF[q] : QBOFF[q] + QBLK[q]],
                                T[:].rearrange("p (b k) -> p b k", k=32),
                                axis=mybir.AxisListType.X,
                                op=mybir.AluOpType.max,
                                apply_absolute_value=True,
                            )
                            Ts[(ct, q)] = T
                        # exponent math for this quarter-pair on [P, bn]
                        m = sm.tile([P, bn], F32, tag="st", name="qm")
                        nc.vector.tensor_scalar(
                            m[:], S[ct][:, ssl], 1e-12, None,
                            op0=mybir.AluOpType.max,
                        )
                        peb = sm.tile([P, bn], I32, tag="st", name="qpeb")
                        nc.vector.tensor_scalar(
                            peb[:], m[:].bitcast(I32), EXP_MASK, None,
                            op0=mybir.AluOpType.bitwise_and,
                        )
                        invb = sm.tile([P, bn], I32, tag="st", name="qinvb")
                        nc.vector.tensor_scalar(
                            invb[:], peb[:], EXP_RSUB, -1.0,
                            op0=mybir.AluOpType.subtract, op1=mybir.AluOpType.mult,
                        )
                        nc.vector.tensor_scalar(
                            inv2[ct][:, ssl], invb[:].bitcast(F32), 128.0, None,
                            op0=mybir.AluOpType.mult,
                        )
                        nc.vector.tensor_scalar(
                            pes[ct][:, ssl], peb[:].bitcast(F32), 0.0078125, None,
                            op0=mybir.AluOpType.mult,
                        )
                    # separate tiles per stage: in-place (same AP in/out)
                    # measures 2-4x slower — it defeats the DVE/GPSIMD
                    # perf modes
                    if stage_major:
                        chain = [(ct, q) for ct in range(CIN_T) for q in pair]
                    else:
                        # img 0: q-major so both cin-halves of the earlier
                        # quarter finish first (conv groups consume quarters
                        # in order)
                        chain = [(ct, q) for q in pair for ct in range(CIN_T)]
                    Vs, R2s, Cs, Qs = {}, {}, {}, {}

                    def st_v(ct, q):
                        wlen, nb = QUARTERS[q], QBLK[q]
                        bsl = slice(QBOFF[q], QBOFF[q] + nb)
                        v = qf.tile([P, wlen], F32, tag="q", name="qv")
                        nc.gpsimd.tensor_tensor(
                            out=v[:].rearrange("p (b k) -> p b k", k=32),
                            in0=Ts[(ct, q)][:].rearrange("p (b k) -> p b k", k=32),
                            in1=inv2[ct][:, bsl].unsqueeze(2).to_broadcast(
                                (P, nb, 32)
                            ),
                            op=mybir.AluOpType.mult,
                        )
                        Vs[(ct, q)] = v

                    def st_r2(ct, q):
                        r2 = qf.tile([P, QUARTERS[q]], F32, tag="q", name="qr2")
                        nc.vector.tensor_scalar(
                            r2[:], Vs[(ct, q)][:], MAGIC, -MAGIC,
                            op0=mybir.AluOpType.add, op1=mybir.AluOpType.add,
                        )
                        R2s[(ct, q)] = r2

                    def st_c(ct, q):
                        c = qf.tile([P, QUARTERS[q]], F32, tag="q", name="qc")
                        nc.gpsimd.tensor_scalar(
                            c[:], R2s[(ct, q)][:], 127.0, -128.0,
                            op0=mybir.AluOpType.min, op1=mybir.AluOpType.max,
                        )
                        Cs[(ct, q)] = c

                    def st_qT(ct, q):
                        wlen, nb = QUARTERS[q], QBLK[q]
                        bsl = slice(QBOFF[q], QBOFF[q] + nb)
                        qT = qb.tile([P, wlen], BF16, tag="qb", name="qq")
                        eng = nc.vector if q % 2 == 0 else nc.gpsimd
                        eng.tensor_tensor(
                            out=qT[:].rearrange("p (b k) -> p b k", k=32),
                            in0=Cs[(ct, q)][:].rearrange("p (b k) -> p b k", k=32),
                            in1=pes[ct][:, bsl].unsqueeze(2).to_broadcast(
                                (P, nb, 32)
                            ),
                            op=mybir.AluOpType.mult,
                        )
                        Qs[(ct, q)] = qT

                    def st_out(ct, q):
                        w0 = QW0 + QOFFS[q]
                        nc.vector.transpose(
                            xq[phz][ct][:, w0 : w0 + QUARTERS[q]], Qs[(ct, q)][:]
                        )

                    stages = (st_v, st_r2, st_c, st_qT, st_out)
                    if stage_major:
                        for f in stages:
                            for ct, q in chain:
                                f(ct, q)
                    else:
                        for ct, q in chain:
                            for f in stages:
                                f(ct, q)

            def emit_dma(img):
                phz = img % NPHASE
                for ct in range(CIN_T):
                    nc.sync.dma_start(
                        out=dst_rows(xpad[phz][ct], 0, H),
                        in_=x_d.ap()[img, ct * P : (ct + 1) * P].rearrange(
                            "c h w -> c (h w)"
                        ),
                    )

            # ---- conv ----
            # img 0: small groups, group-outer/channel-inner, so early
            # groups only need the first quantize quarter-pair and late
            # groups buy time for pair 1.  imgs 1+: 4-chunk groups amortize
            # weight switches across 4 matmuls.
            GROUPS_HEAD = [(0, 1), (2, 3), (4, 5), (6,)]
            GROUPS_STEADY = [(0, 1), (2, 3), (4, 5), (6,)]

            USE_CRIT = False  # critical-section matmul blocks: measured much
            # slower (456us vs 335us) — section entry/exit serialization
            # outweighs the per-MM semaphore savings

            def emit_conv(img):
                phz = img % NPHASE
                groups = GROUPS_HEAD if img == 0 else GROUPS_STEADY
                for grp in groups:
                    for ch in range(COUT_H):
                        pss = {
                            chunk: ps_pool.tile(
                                [P, CHUNK_N], F32, tag="ps", name=f"ps{chunk}"
                            )
                            for chunk in grp
                        }

                        def mm_block():
                            for kt in range(CIN_T):
                                for tap in range(TAPS):
                                    kh, kw = divmod(tap, 3)
                                    acc_i = kt * TAPS + tap
                                    lhsT = wv[:, tap, kt, ch * P : (ch + 1) * P]
                                    for chunk in grp:
                                        base = (
                                            chunk * ROWS_PER_CHUNK + kh
                                        ) * HP + kw
                                        rhs = (
                                            xq[phz][kt][
                                                :, base : base + ROWS_PER_CHUNK * HP
                                            ]
                                            .rearrange(
                                                "p (r w) -> p r w",
                                                r=ROWS_PER_CHUNK,
                                            )[:, :, :W_SP]
                                        )
                                        nc.tensor.matmul(
                                            pss[chunk][:],
                                            lhsT,
                                            rhs,
                                            start=(acc_i == 0),
                                            stop=(acc_i == 2 * TAPS - 1),
                                        )

                        if USE_CRIT and img > 0:
                            with tc.tile_critical(name=f"mm{img}_{ch}"):
                                mm_block()
                        else:
                            mm_block()
                        for chunk in grp:
                            idx = img * NCHUNK + chunk
                            ysl = ybuf[ch][
                                :, img * SPATIAL + chunk * CHUNK_N :
                                img * SPATIAL + (chunk + 1) * CHUNK_N
                            ]
                            nc.scalar.activation(
                                ysl, pss[chunk][:],
                                mybir.ActivationFunctionType.Copy,
                                accum_out=ssum[ch][:, idx : idx + 1],
                            )
                            scr = sm.tile([P, CHUNK_N], BF16, tag="scr", name="sq")
                            nc.scalar.activation(
                                scr[:], pss[chunk][:],
                                mybir.ActivationFunctionType.Square,
                                accum_out=ssq[ch][:, idx : idx + 1],
                            )

            # ---- main loop (software-pipelined) ----
            emit_quantize(0)
            for img in range(B):
                if img + 1 < B:
                    emit_dma(img + 1)
                    emit_quantize(img + 1)
                emit_conv(img)

            # ---- BN statistics reduce ----
            # layout: [sum_ch0, sum_ch1, sumsq_ch0, sumsq_ch1]
            sums_all = pp.tile([P, 4], F32, tag="sums_all")
            for ch in range(COUT_H):
                nc.vector.tensor_reduce(
                    sums_all[:, ch : ch + 1], ssum[ch][:],
                    axis=mybir.AxisListType.X, op=mybir.AluOpType.add,
                )
                nc.vector.tensor_reduce(
                    sums_all[:, 2 + ch : 3 + ch], ssq[ch][:],
                    axis=mybir.AxisListType.X, op=mybir.AluOpType.add,
                )
            gsum = tp.tile([P, 4], F32, tag="t4", name="gsum")
            cc_in = dramp.tile([P, 4], F32)
            cc_out = dramp.tile([P, 4], F32)
            nc.sync.dma_start(out=cc_in[:], in_=sums_all[:])
            nc.gpsimd.collective_compute(
                "AllReduce",
                mybir.AluOpType.add,
                replica_groups=[list(range(n_cores))],
                ins=[cc_in[:].opt()],
                outs=[cc_out[:].opt()],
            )
            nc.sync.dma_start(out=gsum[:], in_=cc_out[:])

            # ---- global scale/shift, both channel halves at once [P,2] ----
            gmean = tp.tile([P, 2], F32, tag="t2")
            nc.vector.tensor_scalar(
                gmean[:], gsum[:, 0:2], 1.0 / n_total, None,
                op0=mybir.AluOpType.mult,
            )
            gex2 = tp.tile([P, 2], F32, tag="t2")
            nc.vector.tensor_scalar(
                gex2[:], gsum[:, 2:4], 1.0 / n_total, None,
                op0=mybir.AluOpType.mult,
            )
            gm2 = tp.tile([P, 2], F32, tag="t2")
            nc.vector.tensor_tensor(gm2[:], gmean[:], gmean[:], op=mybir.AluOpType.mult)
            vdiff = tp.tile([P, 2], F32, tag="t2")
            nc.vector.tensor_tensor(
                vdiff[:], gex2[:], gm2[:], op=mybir.AluOpType.subtract
            )
            veps = tp.tile([P, 2], F32, tag="t2")  # var + eps
            nc.vector.tensor_scalar(
                veps[:], vdiff[:], 1e-5, None, op0=mybir.AluOpType.add
            )
            rec = tp.tile([P, 2], F32, tag="t2")  # 1/(var+eps)
            nc.vector.reciprocal(rec[:], veps[:])
            s0 = tp.tile([P, 2], F32, tag="t2")   # ~= 1/sqrt(var+eps)
            nc.scalar.activation(s0[:], rec[:], mybir.ActivationFunctionType.Sqrt)
            # one Newton step: s1 = s0 * (1.5 - 0.5 * veps * s0^2)
            a = tp.tile([P, 2], F32, tag="t2")
            nc.vector.tensor_tensor(a[:], s0[:], s0[:], op=mybir.AluOpType.mult)
            b = tp.tile([P, 2], F32, tag="t2")
            nc.vector.tensor_tensor(b[:], a[:], veps[:], op=mybir.AluOpType.mult)
            bb = tp.tile([P, 2], F32, tag="t2")
            nc.vector.tensor_scalar(
                bb[:], b[:], -0.5, 1.5,
                op0=mybir.AluOpType.mult, op1=mybir.AluOpType.add,
            )
            s1 = tp.tile([P, 2], F32, tag="t2")
            nc.vector.tensor_tensor(s1[:], s0[:], bb[:], op=mybir.AluOpType.mult)
            scale = tp.tile([P, 2], F32, tag="sc", name="scale")
            nc.vector.tensor_tensor(
                scale[:], s1[:], gbsb[:, 0:2], op=mybir.AluOpType.mult
            )
            t2 = tp.tile([P, 2], F32, tag="t2")
            nc.vector.tensor_tensor(t2[:], gmean[:], scale[:], op=mybir.AluOpType.mult)
            shift = tp.tile([P, 2], F32, tag="sc", name="shift")
            nc.vector.tensor_tensor(
                shift[:], gbsb[:, 2:4], t2[:], op=mybir.AluOpType.subtract
            )
            # u = -shift/scale for the DVE relu max-trick
            rs = tp.tile([P, 2], F32, tag="t2")
            nc.vector.reciprocal(rs[:], scale[:])
            nshift = tp.tile([P, 2], F32, tag="t2")
            nc.vector.tensor_scalar(
                nshift[:], shift[:], -1.0, None, op0=mybir.AluOpType.mult
            )
            u = tp.tile([P, 2], F32, tag="sc", name="uthr")
            nc.vector.tensor_tensor(u[:], nshift[:], rs[:], op=mybir.AluOpType.mult)

            # ---- apply BN + ReLU, write out (bf16) ----
            # DVE takes 5 image-halves via the max-trick (fp16 2x mode,
            # ~2.2us each); ScalarE takes 3 (~3.0us each).
            DVE_APPLY = {(0, 0), (0, 1), (1, 0), (1, 1), (2, 0)}
            for img in range(B):
                for ch in range(COUT_H):
                    ysl = ybuf[ch][:, img * SPATIAL : (img + 1) * SPATIAL]
                    o = op_.tile([P, SPATIAL], BF16, tag="o", name="ostage")
                    if (img, ch) in DVE_APPLY:
                        # s*max(y, u) in place into the dead ybuf slice
                        # (s > 0 since gamma > 0), then + shift
                        nc.vector.tensor_scalar(
                            ysl, ysl, u[:, ch : ch + 1], scale[:, ch : ch + 1],
                            op0=mybir.AluOpType.max, op1=mybir.AluOpType.mult,
                        )
                        nc.vector.tensor_scalar(
                            o[:], ysl, shift[:, ch : ch + 1], None,
                            op0=mybir.AluOpType.add,
                        )
                    else:
                        nc.scalar.activation(
                            o[:], ysl,
                            mybir.ActivationFunctionType.Relu,
                            bias=shift[:, ch : ch + 1],
                            scale=scale[:, ch : ch + 1],
                        )
                    nc.sync.dma_start(
                        out=out_d.ap()[img, ch * P : (ch + 1) * P].rearrange(
                            "c h w -> c (h w)"
                        ),
                        in_=o[:],
                    )

    nc.compile()
    return nc


def host_prep(W, gamma, beta):
    # lhsT layout per tap: [cin, cout];  wt[t, kt, p, o] = W[o, kt*128+p, kh, kw]
    wt = np.ascontiguousarray(
        W.transpose(2, 3, 1, 0).reshape(TAPS, CIN_T, P, 256)
    ).astype(ml_dtypes.bfloat16)
    gb = np.empty((P, 4), np.float32)
    gb[:, 0] = gamma[:P]
    gb[:, 1] = gamma[P:]
    gb[:, 2] = beta[:P]
    gb[:, 3] = beta[P:]
    return wt, gb


_cache = {}


def _get_program(n_cores, imgs_per_core):
    key = (n_cores, imgs_per_core)
    if key not in _cache:
        _cache[key] = build_program(n_cores, imgs_per_core)
    return _cache[key]


def run(x, W, gamma, beta, n_cores=8, trace=False):
    B = x.shape[0]
    imgs_per_core = B // n_cores
    assert imgs_per_core * n_cores == B
    nc = _get_program(n_cores, imgs_per_core)
    wt, gb = host_prep(W, gamma, beta)
    in_maps = [
        {
            "x": np.ascontiguousarray(
                x[c * imgs_per_core : (c + 1) * imgs_per_core]
            ),
            "wt": wt,
            "gb": gb,
        }
        for c in range(n_cores)
    ]
    res = run_bass_kernel_spmd(nc, in_maps, list(range(n_cores)), trace=trace)
    out = np.concatenate(
        [res.results[c]["out"].astype(np.float32) for c in range(n_cores)], axis=0
    )
    return out, res


def kernel(x, W, gamma, beta):
    out, _ = run(
        np.asarray(x, np.float32),
        np.asarray(W, np.float32),
        np.asarray(gamma, np.float32),
        np.asarray(beta, np.float32),
    )
    return out
